# revision 1
# baseline (speedup 1.0000x reference)
"""PolyMPNN Trainium2 kernel: 4-layer edge-MLP message passing GNN.

Strategy (8 NeuronCores, SPMD single program):
- Nodes sharded contiguously: 6272/core (50176 padded). Each core owns the
  edges whose destination (row) falls in its shard, grouped by 128-node
  windows, split by col-half (int16 gather index limit), padded to 128-edge
  chunks with a chunk schedule uniform across cores.
- Per layer: node-parallel matmuls produce P = h@W_r + b1 (local gather
  table) and Q = h@W_c (AllGathered to all cores). Edge phase gathers
  P[row], Q[col] with dma_gather, adds the edge-feature term via a K=2
  matmul in PSUM, relu -> message; scatter-add by one-hot matmul
  (aggT[65,128] += msg[128e,65].T @ onehot[128e,128n]); row 64 (ones col)
  yields per-node degree for the b2 term.
- Node update: h' = relu(LN(aggpre@W2 + deg*b2 + skip_b + h@skip_w)),
  LN in feature-on-partition layout using ones-matmul statistics.
"""
import sys

if "/opt/trn_rl_repo" not in sys.path:
    sys.path.insert(0, "/opt/trn_rl_repo")

import numpy as np

NCORES = 8
N = 50000
NPAD = 50176          # 8 * 6272
NSH = NPAD // NCORES  # 6272 nodes per core
GW = 128              # node group width
G = NSH // GW         # 49 groups per core
HALF = NPAD // 2      # 25088: q table half split (int16 idx limit)
F = 64                # embed
HID = 128             # encoder hidden
L = 4
POLY = 8
TN = 512              # node tile width for matmul passes
GB = 2                # groups per gather batch


def _wrap_idx(idx_flat: np.ndarray) -> np.ndarray:
    """[n] -> [128, n//16] int16 wrapped (16-lane) + replicated layout."""
    n = len(idx_flat)
    assert n % 16 == 0
    a = idx_flat.reshape(n // 16, 16).T.astype(np.int16)
    return np.ascontiguousarray(np.tile(a, (8, 1)))


def _preprocess(node_features, edge_index, edge_features):
    """Sort/pad edges; build per-core device arrays + shared chunk schedule."""
    rows = edge_index[0].astype(np.int64)
    cols = edge_index[1].astype(np.int64)
    E = rows.shape[0]

    owner = rows // NSH
    lrow = rows % NSH
    grp = lrow // GW
    half = (cols >= HALF).astype(np.int64)

    # counts[c, g, h]
    counts = np.zeros((NCORES, G, 2), np.int64)
    np.add.at(counts, (owner, grp, half), 1)
    Klo = np.maximum(np.ceil(counts[:, :, 0].max(axis=0) / 128).astype(np.int64), 0)
    Khi = np.maximum(np.ceil(counts[:, :, 1].max(axis=0) / 128).astype(np.int64), 0)
    K = Klo + Khi                      # chunks per group
    C = int(K.sum())                   # chunks per core (uniform)

    # batches of GB groups; chunk order within batch: lo chunks of each
    # group in batch order, then hi chunks of each group.
    batches = []  # list of dict(groups, c0, kb, klo_b)
    c0 = 0
    for b0 in range(0, G, GB):
        gs = list(range(b0, min(b0 + GB, G)))
        klo_b = int(Klo[gs].sum())
        kb = int(K[gs].sum())
        # chunk positions per group (relative to batch start)
        lopos, hipos = {}, {}
        lo_off, hi_off = 0, klo_b
        for g in gs:
            lopos[g] = (lo_off, lo_off + int(Klo[g]))
            hipos[g] = (hi_off, hi_off + int(Khi[g]))
            lo_off += int(Klo[g])
            hi_off += int(Khi[g])
        batches.append(dict(groups=gs, c0=c0, kb=kb, klo_b=klo_b,
                            lopos=lopos, hipos=hipos))
        c0 += kb
    assert c0 == C

    # order edges per (core, group, half); then fill slot arrays
    order = np.lexsort((half, grp, owner))
    srows, scols, sgrp, sowner, shalf = (lrow[order], cols[order], grp[order],
                                         owner[order], half[order])
    sef = edge_features[order].astype(np.float32)

    # per-core slot arrays
    pidx = np.zeros((NCORES, C * 128), np.int64)        # local row idx
    qidx = np.zeros((NCORES, C * 128), np.int64)        # col idx (half-rel)
    rloc = np.full((NCORES, 128, C), 999.0, np.float32)  # row-in-group or 999
    ef = np.zeros((NCORES, 2, C * 128), np.float32)

    # slot base position for each (core, group, half) in the core's chunk seq
    slot_base = np.zeros((NCORES, G, 2), np.int64)
    for b in batches:
        for g in b["groups"]:
            lo0, _ = b["lopos"][g]
            hi0, _ = b["hipos"][g]
            slot_base[:, g, 0] = (b["c0"] + lo0) * 128
            slot_base[:, g, 1] = (b["c0"] + hi0) * 128

    # scatter edges into slots: rank within (core,group,half)
    key = (sowner * G + sgrp) * 2 + shalf
    # stable sort already groups them; compute rank via running index
    _, first_idx, key_counts = np.unique(key, return_index=True, return_counts=True)
    rank = np.arange(len(key), dtype=np.int64)
    rank -= np.repeat(first_idx, key_counts)
    slot = slot_base[sowner, sgrp, shalf] + rank

    pidx[sowner, slot] = srows
    qidx[sowner, slot] = scols - shalf * HALF
    lane = slot % 128
    chunk = slot // 128
    rloc[sowner, lane, chunk] = (srows % GW).astype(np.float32)
    ef[sowner, 0, slot] = sef[:, 0]
    ef[sowner, 1, slot] = sef[:, 1]

    # wrapped idx arrays per gather call (P: whole batch; Q: lo seg + hi seg)
    pidx_w = np.zeros((NCORES, 128, C * 8), np.int16)
    qidx_w = np.zeros((NCORES, 128, C * 8), np.int16)
    for c in range(NCORES):
        for b in batches:
            s, kb, klo = b["c0"], b["kb"], b["klo_b"]
            pidx_w[c][:, s * 8:(s + kb) * 8] = _wrap_idx(pidx[c][s * 128:(s + kb) * 128])
            if klo > 0:
                qidx_w[c][:, s * 8:(s + klo) * 8] = _wrap_idx(
                    qidx[c][s * 128:(s + klo) * 128])
            if kb - klo > 0:
                qidx_w[c][:, (s + klo) * 8:(s + kb) * 8] = _wrap_idx(
                    qidx[c][(s + klo) * 128:(s + kb) * 128])

    # node features transposed + ones row, per core
    nf = np.zeros((NPAD, 3), np.float32)
    nf[:N] = node_features
    nf1T = np.zeros((NCORES, 4, NSH), np.float32)
    for c in range(NCORES):
        nf1T[c, 0:3] = nf[c * NSH:(c + 1) * NSH].T
        nf1T[c, 3] = 1.0

    sched = dict(Klo=Klo, Khi=Khi, K=K, C=C, batches=batches)
    percore = dict(pidx_w=pidx_w, qidx_w=qidx_w, rloc=rloc, ef=ef, nf1T=nf1T)
    return sched, percore


def _build(nc_mod, sched):
    """Build the Bass program for the shared chunk schedule."""
    import concourse.mybir as mybir
    import concourse.tile as tile
    from concourse import bacc

    dt = mybir.dt
    fp = dt.float32
    AOT = mybir.AluOpType
    ACT = mybir.ActivationFunctionType

    C = sched["C"]
    batches = sched["batches"]
    Klo, Khi, K = sched["Klo"], sched["Khi"], sched["K"]

    nc = bacc.Bacc("TRN2", num_devices=NCORES)

    # ---- I/O ----
    nf1T_d = nc.dram_tensor("nf1T", [4, NSH], fp, kind="ExternalInput")
    pidx_d = nc.dram_tensor("pidx", [128, C * 8], dt.int16, kind="ExternalInput")
    qidx_d = nc.dram_tensor("qidx", [128, C * 8], dt.int16, kind="ExternalInput")
    rloc_d = nc.dram_tensor("rloc", [128, C], fp, kind="ExternalInput")
    ef_d = nc.dram_tensor("ef", [2, C * 128], fp, kind="ExternalInput")
    iota_d = nc.dram_tensor("iota", [128, 128], fp, kind="ExternalInput")
    onesbd_d = nc.dram_tensor("onesbd", [128, 2], fp, kind="ExternalInput")
    ones64_d = nc.dram_tensor("ones64", [1, 64], fp, kind="ExternalInput")
    encw1b_d = nc.dram_tensor("encw1b", [4, HID], fp, kind="ExternalInput")
    encw2_d = nc.dram_tensor("encw2", [HID, F], fp, kind="ExternalInput")
    encb2_d = nc.dram_tensor("encb2", [F, 1], fp, kind="ExternalInput")
    wrb1_d = nc.dram_tensor("wrb1", [L, 65, F], fp, kind="ExternalInput")
    wc_d = nc.dram_tensor("wc", [L, 65, F], fp, kind="ExternalInput")
    web_d = nc.dram_tensor("web", [L, 2, F], fp, kind="ExternalInput")
    w2b_d = nc.dram_tensor("w2b", [L, 65, F], fp, kind="ExternalInput")
    skb_d = nc.dram_tensor("skb", [L, F, 1], fp, kind="ExternalInput")
    skw_d = nc.dram_tensor("skw", [L, F, F], fp, kind="ExternalInput")
    lng_d = nc.dram_tensor("lng", [L, F, 1], fp, kind="ExternalInput")
    lnb_d = nc.dram_tensor("lnb", [L, F, 1], fp, kind="ExternalInput")
    hw1_d = nc.dram_tensor("hw1", [F, F], fp, kind="ExternalInput")
    hb1_d = nc.dram_tensor("hb1", [F, 1], fp, kind="ExternalInput")
    hw2_d = nc.dram_tensor("hw2", [F, POLY], fp, kind="ExternalInput")
    hb2_d = nc.dram_tensor("hb2", [POLY, 1], fp, kind="ExternalInput")
    outT_d = nc.dram_tensor("outT", [POLY, NSH], fp, kind="ExternalOutput")
    # internal
    p_local = nc.dram_tensor("p_local", [NSH, F], fp)
    q_local = nc.dram_tensor("q_local", [NSH, F], fp)
    q_full = nc.dram_tensor("q_full", [NPAD, F], fp)

    ntiles = [(t * TN, min(TN, NSH - t * TN)) for t in range((NSH + TN - 1) // TN)]

    with tile.TileContext(nc) as tc:
        with (
            tc.tile_pool(name="persist", bufs=1) as pp,
            tc.tile_pool(name="wts", bufs=1) as wp,
        ):
            # persistent state
            hT = pp.tile([65, NSH], fp)         # rows 0-63 h, row 64 ones
            aggT = pp.tile([65, NSH], fp)       # rows 0-63 agg, row 64 deg
            iota_t = pp.tile([128, 128], fp)
            onesbd_t = pp.tile([128, 2], fp)
            ones64_t = pp.tile([1, 64], fp)
            nc.sync.dma_start(out=iota_t[:], in_=iota_d[:, :])
            nc.sync.dma_start(out=onesbd_t[:], in_=onesbd_d[:, :])
            nc.sync.dma_start(out=ones64_t[:], in_=ones64_d[:, :])
            nc.vector.memset(hT[64:65, :], 1.0)

            # weights resident
            encw1b_t = wp.tile([4, HID], fp)
            encw2_t = wp.tile([HID, F], fp)
            encb2_t = wp.tile([F, 1], fp)
            nc.sync.dma_start(out=encw1b_t[:], in_=encw1b_d[:, :])
            nc.sync.dma_start(out=encw2_t[:], in_=encw2_d[:, :])
            nc.sync.dma_start(out=encb2_t[:], in_=encb2_d[:, :])
            wrb1_t = [wp.tile([65, F], fp, name=f"wrb1{l}") for l in range(L)]
            wc_t = [wp.tile([65, F], fp, name=f"wc{l}") for l in range(L)]
            web_t = [wp.tile([2, F], fp, name=f"web{l}") for l in range(L)]
            w2b_t = [wp.tile([65, F], fp, name=f"w2b{l}") for l in range(L)]
            skb_t = [wp.tile([F, 1], fp, name=f"skb{l}") for l in range(L)]
            skw_t = [wp.tile([F, F], fp, name=f"skw{l}") for l in range(L)]
            lng_t = [wp.tile([F, 1], fp, name=f"lng{l}") for l in range(L)]
            lnb_t = [wp.tile([F, 1], fp, name=f"lnb{l}") for l in range(L)]
            for l in range(L):
                nc.sync.dma_start(out=wrb1_t[l][:], in_=wrb1_d[l, :, :])
                nc.sync.dma_start(out=wc_t[l][:], in_=wc_d[l, :, :])
                nc.sync.dma_start(out=web_t[l][:], in_=web_d[l, :, :])
                nc.sync.dma_start(out=w2b_t[l][:], in_=w2b_d[l, :, :])
                nc.sync.dma_start(out=skb_t[l][:], in_=skb_d[l, :, :])
                nc.sync.dma_start(out=skw_t[l][:], in_=skw_d[l, :, :])
                nc.sync.dma_start(out=lng_t[l][:], in_=lng_d[l, :, :])
                nc.sync.dma_start(out=lnb_t[l][:], in_=lnb_d[l, :, :])
            hw1_t = wp.tile([F, F], fp)
            hb1_t = wp.tile([F, 1], fp)
            hw2_t = wp.tile([F, POLY], fp)
            hb2_t = wp.tile([POLY, 1], fp)
            nc.sync.dma_start(out=hw1_t[:], in_=hw1_d[:, :])
            nc.sync.dma_start(out=hb1_t[:], in_=hb1_d[:, :])
            nc.sync.dma_start(out=hw2_t[:], in_=hw2_d[:, :])
            nc.sync.dma_start(out=hb2_t[:], in_=hb2_d[:, :])

            # ---------------- encoder ----------------
            with (
                tc.tile_pool(name="enc_sb", bufs=2) as esb,
                tc.tile_pool(name="enc_nf", bufs=1) as enf,
                tc.tile_pool(name="enc_ps", bufs=2, space="PSUM") as eps,
            ):
                nf_t = enf.tile([4, NSH], fp)
                nc.sync.dma_start(out=nf_t[:], in_=nf1T_d[:, :])
                for (t0, tw) in ntiles:
                    hid_ps = eps.tile([HID, TN], fp, tag="hid")
                    nc.tensor.matmul(out=hid_ps[:, :tw], lhsT=encw1b_t[:],
                                     rhs=nf_t[:, t0:t0 + tw], start=True, stop=True)
                    hid_sb = esb.tile([HID, TN], fp, tag="hsb")
                    nc.vector.tensor_scalar(out=hid_sb[:, :tw], in0=hid_ps[:, :tw],
                                            scalar1=0.0, scalar2=None, op0=AOT.max)
                    h_ps = eps.tile([F, TN], fp, tag="hps")
                    nc.tensor.matmul(out=h_ps[:, :tw], lhsT=encw2_t[:],
                                     rhs=hid_sb[:, :tw], start=True, stop=True)
                    nc.vector.tensor_scalar(out=hT[0:F, t0:t0 + tw], in0=h_ps[:, :tw],
                                            scalar1=encb2_t[:, 0:1], scalar2=None,
                                            op0=AOT.add)

            # ---------------- layers ----------------
            for l in range(L):
                # P/Q compute per group
                with (
                    tc.tile_pool(name=f"pq_sb{l}", bufs=3) as qsb,
                    tc.tile_pool(name=f"pq_ps{l}", bufs=3, space="PSUM") as qps,
                ):
                    for g in range(G):
                        sl = slice(g * GW, (g + 1) * GW)
                        pq_ps = qps.tile([GW, 2 * F], fp, tag="pq")
                        nc.tensor.matmul(out=pq_ps[:, 0:F], lhsT=hT[:, sl],
                                         rhs=wrb1_t[l][:], start=True, stop=True)
                        nc.tensor.matmul(out=pq_ps[:, F:2 * F], lhsT=hT[:, sl],
                                         rhs=wc_t[l][:], start=True, stop=True)
                        pq_sb = qsb.tile([GW, 2 * F], fp, tag="pqsb")
                        nc.vector.tensor_copy(out=pq_sb[:], in_=pq_ps[:])
                        nc.sync.dma_start(out=p_local[sl, :], in_=pq_sb[:, 0:F])
                        nc.sync.dma_start(out=q_local[sl, :], in_=pq_sb[:, F:2 * F])

                nc.gpsimd.collective_compute(
                    "AllGather", AOT.bypass,
                    replica_groups=[list(range(NCORES))],
                    ins=[q_local[:, :]], outs=[q_full[:, :]],
                )

                # edge phase
                with (
                    tc.tile_pool(name=f"eg_sb{l}", bufs=2) as gsb,
                    tc.tile_pool(name=f"eg_msg{l}", bufs=2) as msb,
                    tc.tile_pool(name=f"eg_oh{l}", bufs=2) as osb,
                    tc.tile_pool(name=f"eg_ps{l}", bufs=2, space="PSUM") as zps,
                    tc.tile_pool(name=f"agg_ps{l}", bufs=4, space="PSUM") as aps,
                ):
                    for b in batches:
                        kb, klo, s = b["kb"], b["klo_b"], b["c0"]
                        pidx_t = gsb.tile([128, kb * 8], dt.int16, tag="pidx")
                        qidx_t = gsb.tile([128, kb * 8], dt.int16, tag="qidx")
                        rloc_t = gsb.tile([128, kb], fp, tag="rloc")
                        nc.sync.dma_start(out=pidx_t[:, :], in_=pidx_d[:, s * 8:(s + kb) * 8])
                        nc.sync.dma_start(out=qidx_t[:, :], in_=qidx_d[:, s * 8:(s + kb) * 8])
                        nc.sync.dma_start(out=rloc_t[:, :], in_=rloc_d[:, s:s + kb])

                        pg = gsb.tile([128, kb, F], fp, tag="pg")
                        qg = gsb.tile([128, kb, F], fp, tag="qg")
                        nc.gpsimd.dma_gather(
                            out_ap=pg[:], in_ap=p_local[:, :], idxs_ap=pidx_t[:],
                            num_idxs=kb * 128, num_idxs_reg=kb * 128, elem_size=F, single_packet=False)
                        if klo > 0:
                            nc.gpsimd.dma_gather(
                                out_ap=qg[:, 0:klo, :], in_ap=q_full[0:HALF, :],
                                idxs_ap=qidx_t[:, 0:klo * 8],
                                num_idxs=klo * 128, num_idxs_reg=klo * 128,
                                elem_size=F, single_packet=False)
                        if kb - klo > 0:
                            nc.gpsimd.dma_gather(
                                out_ap=qg[:, klo:kb, :], in_ap=q_full[HALF:NPAD, :],
                                idxs_ap=qidx_t[:, klo * 8:kb * 8],
                                num_idxs=(kb - klo) * 128,
                                num_idxs_reg=(kb - klo) * 128, elem_size=F,
                                single_packet=False)

                        # msg = P + Q over whole batch (cols 0:F)
                        msg_t = msb.tile([128, kb, F + 1], fp, tag="msg")
                        nc.vector.memset(msg_t[:, :, F:F + 1], 1.0)
                        nc.vector.tensor_tensor(out=msg_t[:, :, 0:F], in0=pg[:],
                                                in1=qg[:], op=AOT.add)

                        # EF matmuls in slabs of 8 chunks -> psum [128, 512]
                        nslab = (kb + 7) // 8
                        for si in range(nslab):
                            sc0 = si * 8
                            scw = min(8, kb - sc0)
                            ef_t = gsb.tile([2, 8 * 128], fp, tag="ef")
                            nc.sync.dma_start(
                                out=ef_t[:, 0:scw * 128],
                                in_=ef_d[:, (s + sc0) * 128:(s + sc0 + scw) * 128])
                            z_ps = zps.tile([128, 512], fp, tag="z")
                            for c in range(sc0, sc0 + scw):
                                nc.tensor.matmul(
                                    out=z_ps[:, (c - sc0) * F:(c - sc0 + 1) * F],
                                    lhsT=ef_t[:, (c - sc0) * 128:(c - sc0 + 1) * 128],
                                    rhs=web_t[l][:], start=True, stop=True)
                            # msg += z ; msg = relu(msg)
                            nc.vector.tensor_tensor(
                                out=msg_t[:, sc0:sc0 + scw, 0:F],
                                in0=msg_t[:, sc0:sc0 + scw, 0:F],
                                in1=z_ps[:, 0:scw * F].rearrange(
                                    "p (c f) -> p c f", f=F),
                                op=AOT.add)
                            nc.scalar.activation(
                                out=msg_t[:, sc0:sc0 + scw, 0:F],
                                in_=msg_t[:, sc0:sc0 + scw, 0:F], func=ACT.Relu)

                        # onehot + scatter per group
                        for g in b["groups"]:
                            kg = int(K[g])
                            if kg == 0:
                                continue
                            # group chunk ranges (relative to batch)
                            lo0, lo1 = b["lopos"][g]
                            hi0, hi1 = b["hipos"][g]
                            ranges = [(lo0, lo1), (hi0, hi1)]
                            ranges = [(a, z) for (a, z) in ranges if z > a]
                            oh_t = osb.tile([128, kg, 128], fp, tag="oh")
                            ohofs = 0
                            for (a, z) in ranges:
                                nc.vector.tensor_tensor(
                                    out=oh_t[:, ohofs:ohofs + (z - a), :],
                                    in0=rloc_t[:, a:z, None].to_broadcast(
                                        [128, z - a, 128]),
                                    in1=iota_t[:, None, :].to_broadcast(
                                        [128, z - a, 128]),
                                    op=AOT.is_equal)
                                ohofs += z - a
                            agg_ps = aps.tile([F + 1, GW], fp, tag="agg")
                            ci = 0
                            for (a, z) in ranges:
                                for c in range(a, z):
                                    nc.tensor.matmul(
                                        out=agg_ps[:],
                                        lhsT=msg_t[:, c, :],
                                        rhs=oh_t[:, ci, :],
                                        start=(ci == 0), stop=(ci == kg - 1))
                                    ci += 1
                            nc.vector.tensor_copy(
                                out=aggT[0:F + 1, g * GW:(g + 1) * GW],
                                in_=agg_ps[:])

                # node update + LN + relu
                with (
                    tc.tile_pool(name=f"nu_sb{l}", bufs=2) as nsb,
                    tc.tile_pool(name=f"nu_ps{l}", bufs=2, space="PSUM") as nps,
                    tc.tile_pool(name=f"nu_ps2{l}", bufs=1, space="PSUM") as nps2,
                ):
                    for (t0, tw) in ntiles:
                        sl = slice(t0, t0 + tw)
                        hn_ps = nps.tile([F, TN], fp, tag="hn")
                        nc.tensor.matmul(out=hn_ps[:, :tw], lhsT=w2b_t[l][:],
                                         rhs=aggT[:, sl], start=True, stop=False)
                        nc.tensor.matmul(out=hn_ps[:, :tw], lhsT=skw_t[l][:],
                                         rhs=hT[0:F, sl], start=False, stop=True)
                        # LN stats
                        xsq = nsb.tile([128, TN], fp, tag="xsq")
                        nc.vector.tensor_scalar(out=xsq[0:F, :tw], in0=hn_ps[:, :tw],
                                                scalar1=skb_t[l][:, 0:1], scalar2=None,
                                                op0=AOT.add)
                        nc.vector.tensor_tensor(out=xsq[F:2 * F, :tw],
                                                in0=xsq[0:F, :tw], in1=xsq[0:F, :tw],
                                                op=AOT.mult)
                        stmu_ps = nps2.tile([1, TN], fp, tag="stmu")
                        stm2_ps = nps2.tile([1, TN], fp, tag="stm2")
                        nc.tensor.matmul(out=stmu_ps[:, :tw], lhsT=onesbd_t[:, 0:1],
                                         rhs=xsq[:, :tw], start=True, stop=True)
                        nc.tensor.matmul(out=stm2_ps[:, :tw], lhsT=onesbd_t[:, 1:2],
                                         rhs=xsq[:, :tw], start=True, stop=True)
                        # mu, m2, var, rstd, tneg
                        murow = nsb.tile([1, TN], fp, tag="murow")
                        m2row = nsb.tile([1, TN], fp, tag="m2row")
                        srow = nsb.tile([1, TN], fp, tag="srow")
                        trow = nsb.tile([1, TN], fp, tag="trow")
                        nc.vector.tensor_scalar(out=murow[:, :tw], in0=stmu_ps[:, :tw],
                                                scalar1=1.0 / F, scalar2=None,
                                                op0=AOT.mult)
                        nc.vector.tensor_scalar(out=m2row[:, :tw], in0=stm2_ps[:, :tw],
                                                scalar1=1.0 / F, scalar2=None,
                                                op0=AOT.mult)
                        # srow = var = m2 - mu^2
                        nc.vector.scalar_tensor_tensor(
                            out=srow[:, :tw], in0=murow[:, :tw], scalar=-1.0,
                            in1=murow[:, :tw], op0=AOT.mult, op1=AOT.mult)
                        nc.vector.tensor_tensor(out=srow[:, :tw],
                                                in0=srow[:, :tw],
                                                in1=m2row[:, :tw], op=AOT.add)
                        nc.vector.tensor_scalar(out=srow[:, :tw],
                                                in0=srow[:, :tw], scalar1=1e-5,
                                                scalar2=None, op0=AOT.add)
                        nc.scalar.activation(out=srow[:, :tw], in_=srow[:, :tw],
                                             func=ACT.Sqrt)
                        nc.vector.reciprocal(out=srow[:, :tw], in_=srow[:, :tw])
                        # trow = -mu * rstd
                        nc.vector.scalar_tensor_tensor(
                            out=trow[:, :tw], in0=murow[:, :tw], scalar=-1.0,
                            in1=srow[:, :tw], op0=AOT.mult, op1=AOT.mult)
                        # broadcast s, t via K=1 matmuls
                        sb_ps = nps2.tile([F, TN], fp, tag="sb")
                        tb_ps = nps2.tile([F, TN], fp, tag="tb")
                        nc.tensor.matmul(out=sb_ps[:, :tw], lhsT=ones64_t[:],
                                         rhs=srow[:, :tw], start=True, stop=True)
                        nc.tensor.matmul(out=tb_ps[:, :tw], lhsT=ones64_t[:],
                                         rhs=trow[:, :tw], start=True, stop=True)
                        y = nsb.tile([F, TN], fp, tag="y")
                        nc.vector.tensor_tensor(out=y[:, :tw], in0=xsq[0:F, :tw],
                                                in1=sb_ps[:, :tw], op=AOT.mult)
                        nc.vector.tensor_tensor(out=y[:, :tw], in0=y[:, :tw],
                                                in1=tb_ps[:, :tw], op=AOT.add)
                        # h = relu(y*g + b)
                        nc.scalar.activation(out=hT[0:F, sl], in_=y[:, :tw],
                                             func=ACT.Relu,
                                             bias=lnb_t[l][:, 0:1],
                                             scale=lng_t[l][:, 0:1])

            # ---------------- head ----------------
            with (
                tc.tile_pool(name="hd_sb", bufs=2) as hsb,
                tc.tile_pool(name="hd_ps", bufs=2, space="PSUM") as hps,
            ):
                for (t0, tw) in ntiles:
                    sl = slice(t0, t0 + tw)
                    z_ps = hps.tile([F, TN], fp, tag="z1")
                    nc.tensor.matmul(out=z_ps[:, :tw], lhsT=hw1_t[:],
                                     rhs=hT[0:F, sl], start=True, stop=True)
                    z_sb = hsb.tile([F, TN], fp, tag="z1sb")
                    nc.vector.tensor_scalar(out=z_sb[:, :tw], in0=z_ps[:, :tw],
                                            scalar1=hb1_t[:, 0:1], scalar2=0.0,
                                            op0=AOT.add, op1=AOT.max)
                    o_ps = hps.tile([POLY, TN], fp, tag="ops")
                    nc.tensor.matmul(out=o_ps[:, :tw], lhsT=hw2_t[:],
                                     rhs=z_sb[:, :tw], start=True, stop=True)
                    o_sb = hsb.tile([POLY, TN], fp, tag="osb")
                    nc.vector.tensor_scalar(out=o_sb[:, :tw], in0=o_ps[:, :tw],
                                            scalar1=hb2_t[:, 0:1], scalar2=None,
                                            op0=AOT.add)
                    nc.sync.dma_start(out=outT_d[:, t0:t0 + tw], in_=o_sb[:, :tw])

    nc.compile()
    return nc


def _run(inputs, trace=False):
    from concourse import bass_utils

    node_features = np.asarray(inputs["node_features"], np.float32)
    edge_index = np.asarray(inputs["edge_index"])
    edge_features = np.asarray(inputs["edge_features"], np.float32)

    sched, percore = _preprocess(node_features, edge_index, edge_features)
    nc = _build(None, sched)

    # ---- weights (host prep) ----
    s = np.float32
    enc_w1 = np.asarray(inputs["enc_w1"], s)   # [3, HID]
    enc_b1 = np.asarray(inputs["enc_b1"], s)
    enc_w2 = np.asarray(inputs["enc_w2"], s)   # [HID, F]
    enc_b2 = np.asarray(inputs["enc_b2"], s)
    conv_w1 = np.asarray(inputs["conv_w1"], s)  # [L, 130, F]
    conv_b1 = np.asarray(inputs["conv_b1"], s)  # [L, F]
    conv_w2 = np.asarray(inputs["conv_w2"], s)  # [L, F, F]
    conv_b2 = np.asarray(inputs["conv_b2"], s)  # [L, F]
    skip_w = np.asarray(inputs["skip_w"], s)    # [L, F, F]
    skip_b = np.asarray(inputs["skip_b"], s)    # [L, F]
    ln_g = np.asarray(inputs["ln_g"], s)        # [L, F]
    ln_b = np.asarray(inputs["ln_b"], s)
    head_w1 = np.asarray(inputs["head_w1"], s)
    head_b1 = np.asarray(inputs["head_b1"], s)
    head_w2 = np.asarray(inputs["head_w2"], s)
    head_b2 = np.asarray(inputs["head_b2"], s)

    encw1b = np.concatenate([enc_w1, enc_b1[None, :]], axis=0)          # [4, HID]
    wrb1 = np.concatenate([conv_w1[:, 0:F, :], conv_b1[:, None, :]], axis=1)  # [L,65,F]
    wc = np.concatenate([conv_w1[:, F:2 * F, :],
                         np.zeros((L, 1, F), s)], axis=1)               # [L,65,F]
    web = conv_w1[:, 2 * F:2 * F + 2, :]                                # [L,2,F]
    w2b = np.concatenate([conv_w2, conv_b2[:, None, :]], axis=1)    # [L,65,F]

    iota = np.tile(np.arange(128, dtype=s), (128, 1))
    onesbd = np.zeros((128, 2), s)
    onesbd[0:F, 0] = 1.0
    onesbd[F:2 * F, 1] = 1.0
    ones64 = np.ones((1, F), s)

    shared = dict(
        iota=iota, onesbd=onesbd, ones64=ones64,
        encw1b=encw1b, encw2=enc_w2, encb2=enc_b2.reshape(F, 1),
        wrb1=wrb1, wc=wc, web=web, w2b=w2b, skw=skip_w,
        skb=skip_b.reshape(L, F, 1),
        lng=ln_g.reshape(L, F, 1), lnb=ln_b.reshape(L, F, 1),
        hw1=head_w1, hb1=head_b1.reshape(F, 1),
        hw2=head_w2, hb2=head_b2.reshape(POLY, 1),
    )
    in_maps = []
    for c in range(NCORES):
        m = dict(shared)
        m["nf1T"] = percore["nf1T"][c]
        m["pidx"] = percore["pidx_w"][c]
        m["qidx"] = percore["qidx_w"][c]
        m["rloc"] = percore["rloc"][c]
        m["ef"] = percore["ef"][c]
        in_maps.append(m)

    res = bass_utils.run_bass_kernel_spmd(
        nc, in_maps, core_ids=list(range(NCORES)), trace=trace)
    outs = res.results
    full = np.concatenate([outs[c]["outT"].T for c in range(NCORES)], axis=0)
    return full[:N], res


def kernel(**inputs) -> np.ndarray:
    out, _ = _run(inputs, trace=False)
    return out



# revision 10
# speedup vs baseline: 1.4870x; 1.4870x over previous
"""PolyMPNN Trainium2 kernel: 4-layer edge-MLP message passing GNN.

Strategy (8 NeuronCores, SPMD single program):
- Nodes sharded contiguously: 6300/core (50400 padded), groups of GW=126.
  Each core owns edges whose destination (row) falls in its shard, grouped
  by 126-node windows and class-split by col%4, padded to 128-edge chunks
  with a chunk schedule uniform across cores.
- Per layer, per group one matmul produces [P | Q] = h@[W_r+b1 | W_c].
  Q rows go to DRAM (bf16) and are AllGathered; the Q table is then read
  as packed 4-node rows (512B) so the per-edge dma_gather needs 4x fewer
  indices (gpsimd descriptor-gen is the bottleneck resource).
- P[row]+ef@We is broadcast to edges by a single matmul per chunk whose
  weights are a host-precomputed [onehot^T(126); ef(2)] block (bf16).
- msg = relu(P+efW + Qslice) in bf16; scatter-add by one-hot matmul
  (aggT[65,126] += msg[128e,65].T @ onehot[128e,126]); msg col 64 (ones)
  yields per-node degree for the conv b2 term.
- Node update: h' = relu(LN(agg@W2 + deg*b2 + skip_b + h@skip_w)), done
  feature-on-partition with ones-matmul statistics.
"""
import sys

if "/opt/trn_rl_repo" not in sys.path:
    sys.path.insert(0, "/opt/trn_rl_repo")

import numpy as np

NCORES = 8
N = 50000
GW = 126              # node group width
G = 50                # groups per core
NSH = GW * G          # 6300 nodes per core
NPAD = NCORES * NSH   # 50400
PACK = 4              # nodes per packed Q-table row (class split by col%PACK)
F = 64                # embed
HID = 128             # encoder hidden
L = 4
POLY = 8
TN = 450              # node tile width for matmul passes (6300 = 14*450)
GB = 2                # groups per gather batch


def _wrap_idx(idx_flat: np.ndarray) -> np.ndarray:
    """[n] -> [128, n//16] int16 wrapped (16-lane) + replicated layout."""
    n = len(idx_flat)
    assert n % 16 == 0
    a = idx_flat.reshape(n // 16, 16).T.astype(np.int16)
    return np.ascontiguousarray(np.tile(a, (8, 1)))


def _preprocess(node_features, edge_index, edge_features):
    """Sort/pad edges; build per-core device arrays + shared chunk schedule."""
    rows = edge_index[0].astype(np.int64)
    cols = edge_index[1].astype(np.int64)
    ef = np.asarray(edge_features, np.float32)

    owner = rows // NSH
    lrow = rows % NSH
    grp = lrow // GW
    rl = lrow % GW
    cls = cols % PACK
    qi = cols // PACK

    # counts[c, g, j] -> chunks per (g, j) uniform across cores
    counts = np.zeros((NCORES, G, PACK), np.int64)
    np.add.at(counts, (owner, grp, cls), 1)
    Kgc = np.ceil(counts.max(axis=0) / 128).astype(np.int64)  # [G, PACK]
    K = Kgc.sum(axis=1)                                        # [G]
    C = int(K.sum())

    # batches of GB groups; within a batch chunks are group-major, class-minor
    batches = []
    slot_base = np.zeros((G, PACK), np.int64)  # chunk index of (g, j) start
    c0 = 0
    for b0 in range(0, G, GB):
        gs = list(range(b0, min(b0 + GB, G)))
        kb = int(K[gs].sum())
        cls_of_chunk = []
        gspan = {}  # g -> (start chunk within batch, nchunks)
        off = 0
        for g in gs:
            gspan[g] = (off, int(K[g]))
            for j in range(PACK):
                slot_base[g, j] = (c0 + off) * 128
                cls_of_chunk += [j] * int(Kgc[g, j])
                off += int(Kgc[g, j])
        batches.append(dict(groups=gs, c0=c0, kb=kb, gspan=gspan,
                            cls_of_chunk=cls_of_chunk))
        c0 += kb
    assert c0 == C

    # order edges per (core, group, class); rank within each bucket
    order = np.lexsort((cls, grp, owner))
    so, sg, sc = owner[order], grp[order], cls[order]
    key = (so * G + sg) * PACK + sc
    _, first_idx, key_counts = np.unique(key, return_index=True,
                                         return_counts=True)
    rank = np.arange(len(key), dtype=np.int64)
    rank -= np.repeat(first_idx, key_counts)
    slot = slot_base[sg, sc] + rank              # slot within the core
    srl = rl[order]
    sqi = qi[order]
    sef = ef[order]

    qidx = np.zeros((NCORES, C * 128), np.int64)
    rloc = np.full((NCORES, 128, C), 999.0, np.float32)
    ohTef = np.zeros((NCORES, 128, C * 128), np.float32)

    qidx[so, slot] = sqi
    lane = slot % 128
    chunk = slot // 128
    rloc[so, lane, chunk] = srl.astype(np.float32)
    ohTef[so, srl, slot] = 1.0
    ohTef[so, 126, slot] = sef[:, 0]
    ohTef[so, 127, slot] = sef[:, 1]

    # wrapped idx arrays, one contiguous wrap per batch
    qidx_w = np.zeros((NCORES, 128, C * 8), np.int16)
    for c in range(NCORES):
        for b in batches:
            s, kb = b["c0"], b["kb"]
            qidx_w[c][:, s * 8:(s + kb) * 8] = _wrap_idx(
                qidx[c][s * 128:(s + kb) * 128])

    # node features transposed + ones row, per core
    nf = np.zeros((NPAD, 3), np.float32)
    nf[:N] = np.asarray(node_features, np.float32)
    nf1T = np.zeros((NCORES, 4, NSH), np.float32)
    for c in range(NCORES):
        nf1T[c, 0:3] = nf[c * NSH:(c + 1) * NSH].T
        nf1T[c, 3] = 1.0

    import ml_dtypes
    bf = ml_dtypes.bfloat16
    sched = dict(K=K, C=C, batches=batches)
    percore = dict(qidx_w=qidx_w, rloc=rloc.astype(bf),
                   ohTef=ohTef.astype(bf), nf1T=nf1T)
    return sched, percore


def _build(sched):
    """Build the Bass program for the shared chunk schedule."""
    import concourse.mybir as mybir
    import concourse.tile as tile
    from concourse import bacc

    dt = mybir.dt
    fp = dt.float32
    bf = dt.bfloat16
    AOT = mybir.AluOpType
    ACT = mybir.ActivationFunctionType

    C = sched["C"]
    batches = sched["batches"]
    K = sched["K"]
    kb_max = max(b["kb"] for b in batches)

    nc = bacc.Bacc("TRN2", num_devices=NCORES)

    # ---- I/O ----
    nf1T_d = nc.dram_tensor("nf1T", [4, NSH], fp, kind="ExternalInput")
    qidx_d = nc.dram_tensor("qidx", [128, C * 8], dt.int16, kind="ExternalInput")
    rloc_d = nc.dram_tensor("rloc", [128, C], bf, kind="ExternalInput")
    ohTef_d = nc.dram_tensor("ohTef", [128, C * 128], bf, kind="ExternalInput")
    iota_d = nc.dram_tensor("iota", [128, GW], bf, kind="ExternalInput")
    onesbd_d = nc.dram_tensor("onesbd", [128, 2], fp, kind="ExternalInput")
    ones64_d = nc.dram_tensor("ones64", [1, F], fp, kind="ExternalInput")
    encw1b_d = nc.dram_tensor("encw1b", [4, HID], fp, kind="ExternalInput")
    encw2_d = nc.dram_tensor("encw2", [HID, F], fp, kind="ExternalInput")
    encb2_d = nc.dram_tensor("encb2", [F, 1], fp, kind="ExternalInput")
    wrb1wc_d = nc.dram_tensor("wrb1wc", [L, 65, 2 * F], fp, kind="ExternalInput")
    web_d = nc.dram_tensor("web", [L, 32, F], bf, kind="ExternalInput")
    w2b_d = nc.dram_tensor("w2b", [L, 65, F], fp, kind="ExternalInput")
    skb_d = nc.dram_tensor("skb", [L, F, 1], fp, kind="ExternalInput")
    skw_d = nc.dram_tensor("skw", [L, F, F], fp, kind="ExternalInput")
    lng_d = nc.dram_tensor("lng", [L, F, 1], fp, kind="ExternalInput")
    lnb_d = nc.dram_tensor("lnb", [L, F, 1], fp, kind="ExternalInput")
    hw1_d = nc.dram_tensor("hw1", [F, F], fp, kind="ExternalInput")
    hb1_d = nc.dram_tensor("hb1", [F, 1], fp, kind="ExternalInput")
    hw2_d = nc.dram_tensor("hw2", [F, POLY], fp, kind="ExternalInput")
    hb2_d = nc.dram_tensor("hb2", [POLY, 1], fp, kind="ExternalInput")
    outT_d = nc.dram_tensor("outT", [POLY, NSH], fp, kind="ExternalOutput")
    houtT_d = nc.dram_tensor("houtT", [F, NSH], fp, kind="ExternalOutput")
    # internal
    q_local = nc.dram_tensor("q_local", [NSH, F], bf)
    q_full = nc.dram_tensor("q_full", [NPAD, F], bf, addr_space="Shared")

    ntiles = [(t * TN, TN) for t in range(NSH // TN)]

    with tile.TileContext(nc) as tc:
        with (
            tc.tile_pool(name="persist", bufs=1) as pp,
            tc.tile_pool(name="wts", bufs=1) as wp,
        ):
            # persistent state
            hT = pp.tile([65, NSH], fp)          # rows 0-63 h, row 64 ones
            aggT = pp.tile([65, NSH], fp)        # rows 0-63 agg, row 64 deg
            pq_sb = pp.tile([128, G, 2 * F], bf)  # per group: [P(126)+web | Q]
            iota_t = pp.tile([128, GW], bf)
            onesbd_t = pp.tile([128, 2], fp)
            ones64_t = pp.tile([1, F], fp)
            nc.sync.dma_start(out=iota_t[:], in_=iota_d[:, :])
            nc.sync.dma_start(out=onesbd_t[:], in_=onesbd_d[:, :])
            nc.sync.dma_start(out=ones64_t[:], in_=ones64_d[:, :])
            nc.vector.memset(hT[64:65, :], 1.0)

            # weights resident
            encw1b_t = wp.tile([4, HID], fp)
            encw2_t = wp.tile([HID, F], fp)
            encb2_t = wp.tile([F, 1], fp)
            nc.sync.dma_start(out=encw1b_t[:], in_=encw1b_d[:, :])
            nc.sync.dma_start(out=encw2_t[:], in_=encw2_d[:, :])
            nc.sync.dma_start(out=encb2_t[:], in_=encb2_d[:, :])
            wrb1wc_t = [wp.tile([65, 2 * F], fp, name=f"wrb1wc{l}") for l in range(L)]
            web_t = [wp.tile([32, F], bf, name=f"web{l}") for l in range(L)]
            w2b_t = [wp.tile([65, F], fp, name=f"w2b{l}") for l in range(L)]
            skb_t = [wp.tile([F, 1], fp, name=f"skb{l}") for l in range(L)]
            skw_t = [wp.tile([F, F], fp, name=f"skw{l}") for l in range(L)]
            lng_t = [wp.tile([F, 1], fp, name=f"lng{l}") for l in range(L)]
            lnb_t = [wp.tile([F, 1], fp, name=f"lnb{l}") for l in range(L)]
            for l in range(L):
                nc.sync.dma_start(out=wrb1wc_t[l][:], in_=wrb1wc_d[l, :, :])
                nc.sync.dma_start(out=web_t[l][:], in_=web_d[l, :, :])
                nc.sync.dma_start(out=w2b_t[l][:], in_=w2b_d[l, :, :])
                nc.sync.dma_start(out=skb_t[l][:], in_=skb_d[l, :, :])
                nc.sync.dma_start(out=skw_t[l][:], in_=skw_d[l, :, :])
                nc.sync.dma_start(out=lng_t[l][:], in_=lng_d[l, :, :])
                nc.sync.dma_start(out=lnb_t[l][:], in_=lnb_d[l, :, :])
            hw1_t = wp.tile([F, F], fp)
            hb1_t = wp.tile([F, 1], fp)
            hw2_t = wp.tile([F, POLY], fp)
            hb2_t = wp.tile([POLY, 1], fp)
            nc.sync.dma_start(out=hw1_t[:], in_=hw1_d[:, :])
            nc.sync.dma_start(out=hb1_t[:], in_=hb1_d[:, :])
            nc.sync.dma_start(out=hw2_t[:], in_=hw2_d[:, :])
            nc.sync.dma_start(out=hb2_t[:], in_=hb2_d[:, :])

            # ---------------- encoder ----------------
            with (
                tc.tile_pool(name="enc_sb", bufs=2) as esb,
                tc.tile_pool(name="enc_nf", bufs=1) as enf,
                tc.tile_pool(name="enc_ps", bufs=2, space="PSUM") as eps,
            ):
                nf_t = enf.tile([4, NSH], fp)
                nc.sync.dma_start(out=nf_t[:], in_=nf1T_d[:, :])
                for (t0, tw) in ntiles:
                    hid_ps = eps.tile([HID, TN], fp, tag="hid")
                    nc.tensor.matmul(out=hid_ps[:], lhsT=encw1b_t[:],
                                     rhs=nf_t[:, t0:t0 + tw], start=True, stop=True)
                    hid_sb = esb.tile([HID, TN], fp, tag="hsb")
                    nc.scalar.activation(out=hid_sb[:], in_=hid_ps[:], func=ACT.Relu)
                    h_ps = eps.tile([F, TN], fp, tag="hps")
                    nc.tensor.matmul(out=h_ps[:], lhsT=encw2_t[:],
                                     rhs=hid_sb[:], start=True, stop=True)
                    nc.vector.tensor_scalar(out=hT[0:F, t0:t0 + tw], in0=h_ps[:],
                                            scalar1=encb2_t[:, 0:1], scalar2=None,
                                            op0=AOT.add)

            # ---------------- layers ----------------
            for l in range(L):
                # P/Q compute per group -> pq_sb; Q also to DRAM + AllGather
                with (
                    tc.tile_pool(name=f"pq_ps{l}", bufs=3, space="PSUM") as qps,
                ):
                    # web rows 126:128 for the combined broadcast matmul
                    # (aligned [96:128] write; rows 96:126 re-overwritten by P)
                    nc.vector.tensor_copy(
                        out=pq_sb[96:128, :, 0:F],
                        in_=web_t[l][:, None, :].to_broadcast([32, G, F]))
                    for g in range(G):
                        sl = slice(g * GW, (g + 1) * GW)
                        pq_ps = qps.tile([GW, 2 * F], fp, tag="pq")
                        nc.tensor.matmul(out=pq_ps[:], lhsT=hT[:, sl],
                                         rhs=wrb1wc_t[l][:], start=True, stop=True)
                        nc.vector.tensor_copy(out=pq_sb[0:GW, g, :], in_=pq_ps[:])
                    # Q -> DRAM (node-major), then AllGather
                    nc.sync.dma_start(
                        out=q_local[:, :].rearrange("(g i) f -> i g f", i=GW),
                        in_=pq_sb[0:GW, :, F:2 * F])

                nc.gpsimd.collective_compute(
                    "AllGather", AOT.bypass,
                    replica_groups=[list(range(NCORES))],
                    ins=[q_local[:, :]], outs=[q_full[:, :]],
                )
                qview = q_full[:, :].rearrange("(r k) f -> r (k f)", k=PACK)

                # edge phase
                with (
                    tc.tile_pool(name=f"eg_ix{l}", bufs=3) as ixb,
                    tc.tile_pool(name=f"eg_oh{l}", bufs=3) as ohb,
                    tc.tile_pool(name=f"eg_qg{l}", bufs=2) as qgb,
                    tc.tile_pool(name=f"eg_ms{l}", bufs=3) as msb,
                    tc.tile_pool(name=f"eg_pb{l}", bufs=4, space="PSUM") as pbp,
                    tc.tile_pool(name=f"agg_ps{l}", bufs=2, space="PSUM") as aps,
                ):
                    for b in batches:
                        kb, s = b["kb"], b["c0"]
                        qidx_t = ixb.tile([128, kb_max * 8], dt.int16, tag="qidx")
                        rloc_t = ixb.tile([128, kb_max], bf, tag="rloc")
                        ohTef_t = ixb.tile([128, kb_max * 128], bf, tag="ohTef")
                        nc.sync.dma_start(out=qidx_t[:, 0:kb * 8],
                                          in_=qidx_d[:, s * 8:(s + kb) * 8])
                        nc.sync.dma_start(out=rloc_t[:, 0:kb],
                                          in_=rloc_d[:, s:s + kb])
                        nc.scalar.dma_start(out=ohTef_t[:, 0:kb * 128],
                                            in_=ohTef_d[:, s * 128:(s + kb) * 128])

                        oh_t = ohb.tile([128, kb_max, GW], bf, tag="oh")
                        nc.vector.tensor_tensor(
                            out=oh_t[:, 0:kb, :],
                            in0=rloc_t[:, 0:kb, None].to_broadcast([128, kb, GW]),
                            in1=iota_t[:, None, :].to_broadcast([128, kb, GW]),
                            op=AOT.is_equal)

                        msg_t = msb.tile([128, kb_max, F + 1], bf, tag="msg")
                        nc.vector.memset(msg_t[:, 0:kb, F:F + 1], 1.0)

                        # P + ef@We broadcast into PSUM, one matmul per chunk
                        pb_list = []
                        for ci in range(kb):
                            g = None
                            for gg, (o, k) in b["gspan"].items():
                                if o <= ci < o + k:
                                    g = gg
                                    break
                            pb_ps = pbp.tile([128, F], fp, tag="pb")
                            nc.tensor.matmul(
                                out=pb_ps[:],
                                lhsT=ohTef_t[:, ci * 128:(ci + 1) * 128],
                                rhs=pq_sb[:, g, 0:F], start=True, stop=True)
                            pb_list.append(pb_ps)

                        # gather packed Q rows for the whole batch
                        qg = qgb.tile([128, kb_max, PACK * F], bf, tag="qg")
                        nc.gpsimd.dma_gather(
                            out_ap=qg[:, 0:kb, :], in_ap=qview,
                            idxs_ap=qidx_t[:, 0:kb * 8],
                            num_idxs=kb * 128, num_idxs_reg=kb * 128,
                            elem_size=PACK * F, single_packet=False)

                        # msg = relu(pb + Qslice); scatter per chunk
                        for ci in range(kb):
                            j = b["cls_of_chunk"][ci]
                            nc.vector.tensor_tensor(
                                out=msg_t[:, ci, 0:F], in0=pb_list[ci][:],
                                in1=qg[:, ci, j * F:(j + 1) * F], op=AOT.add)
                            nc.scalar.activation(out=msg_t[:, ci, 0:F],
                                                 in_=msg_t[:, ci, 0:F],
                                                 func=ACT.Relu)

                        for g in b["groups"]:
                            o, k = b["gspan"][g]
                            if k == 0:
                                nc.vector.memset(
                                    aggT[:, g * GW:(g + 1) * GW], 0.0)
                                continue
                            agg_ps = aps.tile([F + 1, GW], fp, tag="agg")
                            for ci in range(o, o + k):
                                nc.tensor.matmul(
                                    out=agg_ps[:],
                                    lhsT=msg_t[:, ci, :],
                                    rhs=oh_t[:, ci, :],
                                    start=(ci == o), stop=(ci == o + k - 1))
                            nc.vector.tensor_copy(
                                out=aggT[:, g * GW:(g + 1) * GW],
                                in_=agg_ps[:])

                # node update + LN + relu
                with (
                    tc.tile_pool(name=f"nu_sb{l}", bufs=2) as nsb,
                    tc.tile_pool(name=f"nu_ps{l}", bufs=2, space="PSUM") as nps,
                    tc.tile_pool(name=f"nu_ps2{l}", bufs=1, space="PSUM") as nps2,
                ):
                    for (t0, tw) in ntiles:
                        sl = slice(t0, t0 + tw)
                        hn_ps = nps.tile([F, TN], fp, tag="hn")
                        nc.tensor.matmul(out=hn_ps[:], lhsT=w2b_t[l][:],
                                         rhs=aggT[:, sl], start=True, stop=False)
                        nc.tensor.matmul(out=hn_ps[:], lhsT=skw_t[l][:],
                                         rhs=hT[0:F, sl], start=False, stop=True)
                        # x rows 0:64, x^2 rows 64:128
                        xsq = nsb.tile([128, TN], fp, tag="xsq")
                        nc.vector.tensor_scalar(out=xsq[0:F, :], in0=hn_ps[:],
                                                scalar1=skb_t[l][:, 0:1],
                                                scalar2=None, op0=AOT.add)
                        nc.vector.tensor_tensor(out=xsq[F:2 * F, :],
                                                in0=xsq[0:F, :], in1=xsq[0:F, :],
                                                op=AOT.mult)
                        stmu_ps = nps2.tile([1, TN], fp, tag="stmu")
                        stm2_ps = nps2.tile([1, TN], fp, tag="stm2")
                        nc.tensor.matmul(out=stmu_ps[:], lhsT=onesbd_t[:, 0:1],
                                         rhs=xsq[:], start=True, stop=True)
                        nc.tensor.matmul(out=stm2_ps[:], lhsT=onesbd_t[:, 1:2],
                                         rhs=xsq[:], start=True, stop=True)
                        murow = nsb.tile([1, TN], fp, tag="murow")
                        m2row = nsb.tile([1, TN], fp, tag="m2row")
                        srow = nsb.tile([1, TN], fp, tag="srow")
                        trow = nsb.tile([1, TN], fp, tag="trow")
                        nc.vector.tensor_scalar(out=murow[:], in0=stmu_ps[:],
                                                scalar1=1.0 / F, scalar2=None,
                                                op0=AOT.mult)
                        nc.vector.tensor_scalar(out=m2row[:], in0=stm2_ps[:],
                                                scalar1=1.0 / F, scalar2=None,
                                                op0=AOT.mult)
                        # srow = var = (-mu)*mu + m2
                        nc.vector.scalar_tensor_tensor(
                            out=srow[:], in0=murow[:], scalar=-1.0,
                            in1=murow[:], op0=AOT.mult, op1=AOT.mult)
                        nc.vector.tensor_tensor(out=srow[:], in0=srow[:],
                                                in1=m2row[:], op=AOT.add)
                        nc.vector.tensor_scalar(out=srow[:], in0=srow[:],
                                                scalar1=1e-5, scalar2=None,
                                                op0=AOT.add)
                        nc.scalar.activation(out=srow[:], in_=srow[:],
                                             func=ACT.Sqrt)
                        nc.vector.reciprocal(out=srow[:], in_=srow[:])
                        nc.vector.scalar_tensor_tensor(
                            out=trow[:], in0=murow[:], scalar=-1.0,
                            in1=srow[:], op0=AOT.mult, op1=AOT.mult)
                        # broadcast rstd, t to F rows via matmuls
                        sb_ps = nps2.tile([F, TN], fp, tag="sb")
                        tb_ps = nps2.tile([F, TN], fp, tag="tb")
                        nc.tensor.matmul(out=sb_ps[:], lhsT=ones64_t[:],
                                         rhs=srow[:], start=True, stop=True)
                        nc.tensor.matmul(out=tb_ps[:], lhsT=ones64_t[:],
                                         rhs=trow[:], start=True, stop=True)
                        y = nsb.tile([F, TN], fp, tag="y")
                        nc.vector.tensor_tensor(out=y[:], in0=xsq[0:F, :],
                                                in1=sb_ps[:], op=AOT.mult)
                        nc.vector.tensor_tensor(out=y[:], in0=y[:],
                                                in1=tb_ps[:], op=AOT.add)
                        nc.scalar.activation(out=hT[0:F, sl], in_=y[:],
                                             func=ACT.Relu,
                                             bias=lnb_t[l][:, 0:1],
                                             scale=lng_t[l][:, 0:1])

            # ---------------- head ----------------
            nc.sync.dma_start(out=houtT_d[:, :], in_=hT[0:F, :])
            with (
                tc.tile_pool(name="hd_sb", bufs=2) as hsb,
                tc.tile_pool(name="hd_ps", bufs=2, space="PSUM") as hps,
            ):
                for (t0, tw) in ntiles:
                    sl = slice(t0, t0 + tw)
                    z_ps = hps.tile([F, TN], fp, tag="z1")
                    nc.tensor.matmul(out=z_ps[:], lhsT=hw1_t[:],
                                     rhs=hT[0:F, sl], start=True, stop=True)
                    z_sb = hsb.tile([F, TN], fp, tag="z1sb")
                    nc.scalar.activation(out=z_sb[:], in_=z_ps[:], func=ACT.Relu,
                                         bias=hb1_t[:, 0:1])
                    o_ps = hps.tile([POLY, TN], fp, tag="ops")
                    nc.tensor.matmul(out=o_ps[:], lhsT=hw2_t[:],
                                     rhs=z_sb[:], start=True, stop=True)
                    o_sb = hsb.tile([POLY, TN], fp, tag="osb")
                    nc.vector.tensor_scalar(out=o_sb[:], in0=o_ps[:],
                                            scalar1=hb2_t[:, 0:1], scalar2=None,
                                            op0=AOT.add)
                    nc.sync.dma_start(out=outT_d[:, t0:t0 + tw], in_=o_sb[:])

    nc.compile()
    return nc


def _host_arrays(inputs):
    s = np.float32
    enc_w1 = np.asarray(inputs["enc_w1"], s)
    enc_b1 = np.asarray(inputs["enc_b1"], s)
    enc_w2 = np.asarray(inputs["enc_w2"], s)
    enc_b2 = np.asarray(inputs["enc_b2"], s)
    conv_w1 = np.asarray(inputs["conv_w1"], s)  # [L, 130, F]
    conv_b1 = np.asarray(inputs["conv_b1"], s)
    conv_w2 = np.asarray(inputs["conv_w2"], s)
    conv_b2 = np.asarray(inputs["conv_b2"], s)
    skip_w = np.asarray(inputs["skip_w"], s)
    skip_b = np.asarray(inputs["skip_b"], s)
    ln_g = np.asarray(inputs["ln_g"], s)
    ln_b = np.asarray(inputs["ln_b"], s)
    head_w1 = np.asarray(inputs["head_w1"], s)
    head_b1 = np.asarray(inputs["head_b1"], s)
    head_w2 = np.asarray(inputs["head_w2"], s)
    head_b2 = np.asarray(inputs["head_b2"], s)

    import ml_dtypes
    bf = ml_dtypes.bfloat16

    encw1b = np.concatenate([enc_w1, enc_b1[None, :]], axis=0)      # [4, HID]
    wrb1 = np.concatenate([conv_w1[:, 0:F, :], conv_b1[:, None, :]], axis=1)
    wc = np.concatenate([conv_w1[:, F:2 * F, :],
                         np.zeros((L, 1, F), s)], axis=1)
    wrb1wc = np.concatenate([wrb1, wc], axis=2)                     # [L,65,128]
    web = np.zeros((L, 32, F), s)                                   # rows 30:32
    web[:, 30:32, :] = conv_w1[:, 2 * F:2 * F + 2, :]
    web = web.astype(bf)
    w2b = np.concatenate([conv_w2, conv_b2[:, None, :]], axis=1)    # [L,65,F]

    iota = np.tile(np.arange(GW, dtype=s), (128, 1)).astype(bf)
    onesbd = np.zeros((128, 2), s)
    onesbd[0:F, 0] = 1.0
    onesbd[F:2 * F, 1] = 1.0
    ones64 = np.ones((1, F), s)

    return dict(
        iota=iota, onesbd=onesbd, ones64=ones64,
        encw1b=encw1b, encw2=enc_w2, encb2=enc_b2.reshape(F, 1),
        wrb1wc=wrb1wc, web=web, w2b=w2b, skw=skip_w,
        skb=skip_b.reshape(L, F, 1),
        lng=ln_g.reshape(L, F, 1), lnb=ln_b.reshape(L, F, 1),
        hw1=head_w1, hb1=head_b1.reshape(F, 1),
        hw2=head_w2, hb2=head_b2.reshape(POLY, 1),
    )


def _run(inputs, trace=False):
    from concourse import bass_utils

    node_features = np.asarray(inputs["node_features"], np.float32)
    edge_index = np.asarray(inputs["edge_index"])
    edge_features = np.asarray(inputs["edge_features"], np.float32)

    sched, percore = _preprocess(node_features, edge_index, edge_features)
    nc = _build(sched)
    shared = _host_arrays(inputs)

    in_maps = []
    for c in range(NCORES):
        m = dict(shared)
        m["nf1T"] = percore["nf1T"][c]
        m["qidx"] = percore["qidx_w"][c]
        m["rloc"] = percore["rloc"][c]
        m["ohTef"] = percore["ohTef"][c]
        in_maps.append(m)

    res = bass_utils.run_bass_kernel_spmd(
        nc, in_maps, core_ids=list(range(NCORES)), trace=trace)
    outs = res.results
    full = np.concatenate([outs[c]["outT"].T for c in range(NCORES)], axis=0)
    return full[:N], res


def kernel(**inputs) -> np.ndarray:
    out, _ = _run(inputs, trace=False)
    return out


# revision 14
# speedup vs baseline: 1.7310x; 1.1641x over previous
"""PolyMPNN Trainium2 kernel: 4-layer edge-MLP message passing GNN.

Strategy (8 NeuronCores, SPMD single program):
- Nodes sharded contiguously: 6300/core (50400 padded), groups of GW=126.
  Each core owns edges whose destination (row) falls in its shard, grouped
  by 126-node windows and class-split by col%4, padded to 128-edge chunks
  with a chunk schedule uniform across cores.
- Per layer, per group one matmul produces [P | Q] = h@[W_r+b1 | W_c].
  Q rows go to DRAM (bf16) and are AllGathered; the Q table is then read
  as packed 4-node rows (512B) so the per-edge dma_gather needs 4x fewer
  indices (gpsimd descriptor-gen is the bottleneck resource).
- P[row]+ef@We is broadcast to edges by a single matmul per chunk whose
  weights are a host-precomputed [onehot^T(126); ef(2)] block (bf16).
- msg = relu(P+efW + Qslice) in bf16; scatter-add by one-hot matmul
  (aggT[65,126] += msg[128e,65].T @ onehot[128e,126]); msg col 64 (ones)
  yields per-node degree for the conv b2 term.
- Node update: h' = relu(LN(agg@W2 + deg*b2 + skip_b + h@skip_w)), done
  feature-on-partition with ones-matmul statistics.
"""
import sys

if "/opt/trn_rl_repo" not in sys.path:
    sys.path.insert(0, "/opt/trn_rl_repo")

import numpy as np

NCORES = 8
N = 50000
GW = 126              # node group width
G = 50                # groups per core
NSH = GW * G          # 6300 nodes per core
NPAD = NCORES * NSH   # 50400
PACK = 2              # nodes per packed Q-table row (class split by col%PACK)
F = 64                # embed
HID = 128             # encoder hidden
L = 4
POLY = 8
TN = 450              # node tile width for matmul passes (6300 = 14*450)
GB = 2                # groups per gather batch


def _wrap_idx(idx_flat: np.ndarray) -> np.ndarray:
    """[n] -> [128, n//16] int16 wrapped (16-lane) + replicated layout."""
    n = len(idx_flat)
    assert n % 16 == 0
    a = idx_flat.reshape(n // 16, 16).T.astype(np.int16)
    return np.ascontiguousarray(np.tile(a, (8, 1)))


def _preprocess(node_features, edge_index, edge_features):
    """Sort/pad edges; build per-core device arrays + shared chunk schedule."""
    rows = edge_index[0].astype(np.int64)
    cols = edge_index[1].astype(np.int64)
    ef = np.asarray(edge_features, np.float32)

    owner = rows // NSH
    lrow = rows % NSH
    grp = lrow // GW
    rl = lrow % GW
    cls = cols % PACK
    qi = cols // PACK

    # counts[c, g, j] -> chunks per (g, j) uniform across cores
    counts = np.zeros((NCORES, G, PACK), np.int64)
    np.add.at(counts, (owner, grp, cls), 1)
    Kgc = np.ceil(counts.max(axis=0) / 128).astype(np.int64)  # [G, PACK]
    K = Kgc.sum(axis=1)                                        # [G]
    C = int(K.sum())

    # batches of GB groups; within a batch chunks are group-major, class-minor
    batches = []
    slot_base = np.zeros((G, PACK), np.int64)  # chunk index of (g, j) start
    c0 = 0
    for b0 in range(0, G, GB):
        gs = list(range(b0, min(b0 + GB, G)))
        kb = int(K[gs].sum())
        cls_of_chunk = []
        gspan = {}  # g -> (start chunk within batch, nchunks)
        off = 0
        for g in gs:
            gspan[g] = (off, int(K[g]))
            for j in range(PACK):
                slot_base[g, j] = (c0 + off) * 128
                cls_of_chunk += [j] * int(Kgc[g, j])
                off += int(Kgc[g, j])
        batches.append(dict(groups=gs, c0=c0, kb=kb, gspan=gspan,
                            cls_of_chunk=cls_of_chunk))
        c0 += kb
    assert c0 == C

    # order edges per (core, group, class); rank within each bucket
    order = np.lexsort((cls, grp, owner))
    so, sg, sc = owner[order], grp[order], cls[order]
    key = (so * G + sg) * PACK + sc
    _, first_idx, key_counts = np.unique(key, return_index=True,
                                         return_counts=True)
    rank = np.arange(len(key), dtype=np.int64)
    rank -= np.repeat(first_idx, key_counts)
    slot = slot_base[sg, sc] + rank              # slot within the core
    srl = rl[order]
    sqi = qi[order]
    sef = ef[order]

    qidx = np.zeros((NCORES, C * 128), np.int64)
    rloc = np.full((NCORES, 128, C), 999.0, np.float32)
    ohTef = np.zeros((NCORES, 128, C * 128), np.float32)

    qidx[so, slot] = sqi
    lane = slot % 128
    chunk = slot // 128
    rloc[so, lane, chunk] = srl.astype(np.float32)
    ohTef[so, srl, slot] = 1.0
    ohTef[so, 126, slot] = sef[:, 0]
    ohTef[so, 127, slot] = sef[:, 1]

    # wrapped idx arrays, one contiguous wrap per batch
    qidx_w = np.zeros((NCORES, 128, C * 8), np.int16)
    for c in range(NCORES):
        for b in batches:
            s, kb = b["c0"], b["kb"]
            qidx_w[c][:, s * 8:(s + kb) * 8] = _wrap_idx(
                qidx[c][s * 128:(s + kb) * 128])

    # node features transposed + ones row, per core
    nf = np.zeros((NPAD, 3), np.float32)
    nf[:N] = np.asarray(node_features, np.float32)
    nf1T = np.zeros((NCORES, 4, NSH), np.float32)
    for c in range(NCORES):
        nf1T[c, 0:3] = nf[c * NSH:(c + 1) * NSH].T
        nf1T[c, 3] = 1.0

    import ml_dtypes
    bf = ml_dtypes.bfloat16
    sched = dict(K=K, C=C, batches=batches)
    percore = dict(qidx_w=qidx_w, rloc=rloc.astype(bf),
                   ohTef=ohTef.astype(bf), nf1T=nf1T)
    return sched, percore


def _build(sched):
    """Build the Bass program for the shared chunk schedule."""
    import concourse.mybir as mybir
    import concourse.tile as tile
    from concourse import bacc

    dt = mybir.dt
    fp = dt.float32
    bf = dt.bfloat16
    AOT = mybir.AluOpType
    ACT = mybir.ActivationFunctionType

    C = sched["C"]
    batches = sched["batches"]
    K = sched["K"]
    kb_max = max(b["kb"] for b in batches)

    nc = bacc.Bacc("TRN2", num_devices=NCORES)

    # ---- I/O ----
    nf1T_d = nc.dram_tensor("nf1T", [4, NSH], fp, kind="ExternalInput")
    qidx_d = nc.dram_tensor("qidx", [128, C * 8], dt.int16, kind="ExternalInput")
    rloc_d = nc.dram_tensor("rloc", [128, C], bf, kind="ExternalInput")
    ohTef_d = nc.dram_tensor("ohTef", [128, C * 128], bf, kind="ExternalInput")
    iota_d = nc.dram_tensor("iota", [128, GW], bf, kind="ExternalInput")
    onesbd_d = nc.dram_tensor("onesbd", [128, 2], fp, kind="ExternalInput")
    ones64_d = nc.dram_tensor("ones64", [1, F], fp, kind="ExternalInput")
    encw1b_d = nc.dram_tensor("encw1b", [4, HID], fp, kind="ExternalInput")
    encw2_d = nc.dram_tensor("encw2", [HID, F], fp, kind="ExternalInput")
    encb2_d = nc.dram_tensor("encb2", [F, 1], fp, kind="ExternalInput")
    wrb1wc_d = nc.dram_tensor("wrb1wc", [L, 65, 2 * F], fp, kind="ExternalInput")
    web_d = nc.dram_tensor("web", [L, 32, F], bf, kind="ExternalInput")
    w2b_d = nc.dram_tensor("w2b", [L, 65, F], fp, kind="ExternalInput")
    skb_d = nc.dram_tensor("skb", [L, F, 1], fp, kind="ExternalInput")
    skw_d = nc.dram_tensor("skw", [L, F, F], fp, kind="ExternalInput")
    lng_d = nc.dram_tensor("lng", [L, F, 1], fp, kind="ExternalInput")
    lnb_d = nc.dram_tensor("lnb", [L, F, 1], fp, kind="ExternalInput")
    hw1_d = nc.dram_tensor("hw1", [F, F], fp, kind="ExternalInput")
    hb1_d = nc.dram_tensor("hb1", [F, 1], fp, kind="ExternalInput")
    hw2_d = nc.dram_tensor("hw2", [F, POLY], fp, kind="ExternalInput")
    hb2_d = nc.dram_tensor("hb2", [POLY, 1], fp, kind="ExternalInput")
    outT_d = nc.dram_tensor("outT", [POLY, NSH], fp, kind="ExternalOutput")
    houtT_d = nc.dram_tensor("houtT", [F, NSH], fp, kind="ExternalOutput")
    # internal
    q_local = nc.dram_tensor("q_local", [NSH, F], bf)
    q_full = nc.dram_tensor("q_full", [NPAD, F], bf, addr_space="Shared")

    ntiles = [(t * TN, TN) for t in range(NSH // TN)]

    with tile.TileContext(nc) as tc:
        with (
            tc.tile_pool(name="persist", bufs=1) as pp,
            tc.tile_pool(name="wts", bufs=1) as wp,
        ):
            # persistent state
            hT = pp.tile([65, NSH], fp)          # rows 0-63 h, row 64 ones
            aggT = pp.tile([65, NSH], fp)        # rows 0-63 agg, row 64 deg
            pq_sb = pp.tile([128, G, 2 * F], bf)  # per group: [P(126)+web | Q]
            iota_t = pp.tile([128, GW], bf)
            onesbd_t = pp.tile([128, 2], fp)
            ones64_t = pp.tile([1, F], fp)
            nc.sync.dma_start(out=iota_t[:], in_=iota_d[:, :])
            nc.sync.dma_start(out=onesbd_t[:], in_=onesbd_d[:, :])
            nc.sync.dma_start(out=ones64_t[:], in_=ones64_d[:, :])
            nc.vector.memset(hT[64:65, :], 1.0)

            # weights resident
            encw1b_t = wp.tile([4, HID], fp)
            encw2_t = wp.tile([HID, F], fp)
            encb2_t = wp.tile([F, 1], fp)
            nc.sync.dma_start(out=encw1b_t[:], in_=encw1b_d[:, :])
            nc.sync.dma_start(out=encw2_t[:], in_=encw2_d[:, :])
            nc.sync.dma_start(out=encb2_t[:], in_=encb2_d[:, :])
            wrb1wc_t = [wp.tile([65, 2 * F], fp, name=f"wrb1wc{l}") for l in range(L)]
            web_t = [wp.tile([32, F], bf, name=f"web{l}") for l in range(L)]
            w2b_t = [wp.tile([65, F], fp, name=f"w2b{l}") for l in range(L)]
            skb_t = [wp.tile([F, 1], fp, name=f"skb{l}") for l in range(L)]
            skw_t = [wp.tile([F, F], fp, name=f"skw{l}") for l in range(L)]
            lng_t = [wp.tile([F, 1], fp, name=f"lng{l}") for l in range(L)]
            lnb_t = [wp.tile([F, 1], fp, name=f"lnb{l}") for l in range(L)]
            for l in range(L):
                nc.sync.dma_start(out=wrb1wc_t[l][:], in_=wrb1wc_d[l, :, :])
                nc.sync.dma_start(out=web_t[l][:], in_=web_d[l, :, :])
                nc.sync.dma_start(out=w2b_t[l][:], in_=w2b_d[l, :, :])
                nc.sync.dma_start(out=skb_t[l][:], in_=skb_d[l, :, :])
                nc.sync.dma_start(out=skw_t[l][:], in_=skw_d[l, :, :])
                nc.sync.dma_start(out=lng_t[l][:], in_=lng_d[l, :, :])
                nc.sync.dma_start(out=lnb_t[l][:], in_=lnb_d[l, :, :])
            hw1_t = wp.tile([F, F], fp)
            hb1_t = wp.tile([F, 1], fp)
            hw2_t = wp.tile([F, POLY], fp)
            hb2_t = wp.tile([POLY, 1], fp)
            nc.sync.dma_start(out=hw1_t[:], in_=hw1_d[:, :])
            nc.sync.dma_start(out=hb1_t[:], in_=hb1_d[:, :])
            nc.sync.dma_start(out=hw2_t[:], in_=hw2_d[:, :])
            nc.sync.dma_start(out=hb2_t[:], in_=hb2_d[:, :])

            # ---------------- encoder ----------------
            with (
                tc.tile_pool(name="enc_sb", bufs=2) as esb,
                tc.tile_pool(name="enc_nf", bufs=1) as enf,
                tc.tile_pool(name="enc_ps", bufs=2, space="PSUM") as eps,
            ):
                nf_t = enf.tile([4, NSH], fp)
                nc.sync.dma_start(out=nf_t[:], in_=nf1T_d[:, :])
                for (t0, tw) in ntiles:
                    hid_ps = eps.tile([HID, TN], fp, tag="hid")
                    nc.tensor.matmul(out=hid_ps[:], lhsT=encw1b_t[:],
                                     rhs=nf_t[:, t0:t0 + tw], start=True, stop=True)
                    hid_sb = esb.tile([HID, TN], fp, tag="hsb")
                    nc.scalar.activation(out=hid_sb[:], in_=hid_ps[:], func=ACT.Relu)
                    h_ps = eps.tile([F, TN], fp, tag="hps")
                    nc.tensor.matmul(out=h_ps[:], lhsT=encw2_t[:],
                                     rhs=hid_sb[:], start=True, stop=True)
                    nc.vector.tensor_scalar(out=hT[0:F, t0:t0 + tw], in0=h_ps[:],
                                            scalar1=encb2_t[:, 0:1], scalar2=None,
                                            op0=AOT.add)

            # ---------------- layers ----------------
            for l in range(L):
                # P/Q compute per group -> pq_sb; Q also to DRAM + AllGather
                with (
                    tc.tile_pool(name=f"pq_ps{l}", bufs=3, space="PSUM") as qps,
                ):
                    # web rows 126:128 for the combined broadcast matmul
                    # (aligned [96:128] write; rows 96:126 re-overwritten by P)
                    nc.vector.tensor_copy(
                        out=pq_sb[96:128, :, 0:F],
                        in_=web_t[l][:, None, :].to_broadcast([32, G, F]))
                    for g in range(G):
                        sl = slice(g * GW, (g + 1) * GW)
                        pq_ps = qps.tile([GW, 2 * F], fp, tag="pq")
                        nc.tensor.matmul(out=pq_ps[:], lhsT=hT[:, sl],
                                         rhs=wrb1wc_t[l][:], start=True, stop=True)
                        nc.vector.tensor_copy(out=pq_sb[0:GW, g, :], in_=pq_ps[:])
                    # Q -> DRAM (node-major), then AllGather
                    nc.sync.dma_start(
                        out=q_local[:, :].rearrange("(g i) f -> i g f", i=GW),
                        in_=pq_sb[0:GW, :, F:2 * F])

                nc.gpsimd.collective_compute(
                    "AllGather", AOT.bypass,
                    replica_groups=[list(range(NCORES))],
                    ins=[q_local[:, :]], outs=[q_full[:, :]],
                )
                qview = q_full[:, :].rearrange("(r k) f -> r (k f)", k=PACK)

                # edge phase
                with (
                    tc.tile_pool(name=f"eg_ix{l}", bufs=3) as ixb,
                    tc.tile_pool(name=f"eg_oh{l}", bufs=3) as ohb,
                    tc.tile_pool(name=f"eg_qg{l}", bufs=3) as qgb,
                    tc.tile_pool(name=f"eg_ms{l}", bufs=3) as msb,
                    tc.tile_pool(name=f"eg_pb{l}", bufs=4, space="PSUM") as pbp,
                    tc.tile_pool(name=f"agg_ps{l}", bufs=2, space="PSUM") as aps,
                ):
                    for b in batches:
                        kb, s = b["kb"], b["c0"]
                        qidx_t = ixb.tile([128, kb_max * 8], dt.int16, tag="qidx")
                        rloc_t = ixb.tile([128, kb_max], bf, tag="rloc")
                        ohTef_t = ixb.tile([128, kb_max * 128], bf, tag="ohTef")
                        nc.sync.dma_start(out=qidx_t[:, 0:kb * 8],
                                          in_=qidx_d[:, s * 8:(s + kb) * 8])
                        nc.sync.dma_start(out=rloc_t[:, 0:kb],
                                          in_=rloc_d[:, s:s + kb])
                        nc.scalar.dma_start(out=ohTef_t[:, 0:kb * 128],
                                            in_=ohTef_d[:, s * 128:(s + kb) * 128])

                        # gather packed Q rows for the whole batch (issue
                        # early: the gpsimd gather stream is the critical
                        # resource and should run back-to-back)
                        qg = qgb.tile([128, kb_max, PACK * F], bf, tag="qg")
                        nc.gpsimd.dma_gather(
                            out_ap=qg[:, 0:kb, :], in_ap=qview,
                            idxs_ap=qidx_t[:, 0:kb * 8],
                            num_idxs=kb * 128, num_idxs_reg=kb * 128,
                            elem_size=PACK * F, single_packet=False)

                        oh_t = ohb.tile([128, kb_max, GW], bf, tag="oh")
                        nc.vector.tensor_tensor(
                            out=oh_t[:, 0:kb, :],
                            in0=rloc_t[:, 0:kb, None].to_broadcast([128, kb, GW]),
                            in1=iota_t[:, None, :].to_broadcast([128, kb, GW]),
                            op=AOT.is_equal)

                        msg_t = msb.tile([128, kb_max, F + 1], bf, tag="msg")
                        nc.vector.memset(msg_t[:, 0:kb, F:F + 1], 1.0)

                        # P + ef@We broadcast into PSUM, one matmul per chunk
                        pb_list = []
                        for ci in range(kb):
                            g = None
                            for gg, (o, k) in b["gspan"].items():
                                if o <= ci < o + k:
                                    g = gg
                                    break
                            pb_ps = pbp.tile([128, F], fp, tag="pb")
                            nc.tensor.matmul(
                                out=pb_ps[:],
                                lhsT=ohTef_t[:, ci * 128:(ci + 1) * 128],
                                rhs=pq_sb[:, g, 0:F], start=True, stop=True)
                            pb_list.append(pb_ps)

                        # msg = relu(pb + Qslice); scatter per chunk
                        for ci in range(kb):
                            j = b["cls_of_chunk"][ci]
                            nc.vector.tensor_tensor(
                                out=msg_t[:, ci, 0:F], in0=pb_list[ci][:],
                                in1=qg[:, ci, j * F:(j + 1) * F], op=AOT.add)
                            nc.scalar.activation(out=msg_t[:, ci, 0:F],
                                                 in_=msg_t[:, ci, 0:F],
                                                 func=ACT.Relu)

                        for g in b["groups"]:
                            o, k = b["gspan"][g]
                            if k == 0:
                                nc.vector.memset(
                                    aggT[:, g * GW:(g + 1) * GW], 0.0)
                                continue
                            agg_ps = aps.tile([F + 1, GW], fp, tag="agg")
                            for ci in range(o, o + k):
                                nc.tensor.matmul(
                                    out=agg_ps[:],
                                    lhsT=msg_t[:, ci, :],
                                    rhs=oh_t[:, ci, :],
                                    start=(ci == o), stop=(ci == o + k - 1))
                            nc.vector.tensor_copy(
                                out=aggT[:, g * GW:(g + 1) * GW],
                                in_=agg_ps[:])

                # node update + LN + relu
                with (
                    tc.tile_pool(name=f"nu_sb{l}", bufs=2) as nsb,
                    tc.tile_pool(name=f"nu_ps{l}", bufs=2, space="PSUM") as nps,
                    tc.tile_pool(name=f"nu_ps2{l}", bufs=1, space="PSUM") as nps2,
                ):
                    for (t0, tw) in ntiles:
                        sl = slice(t0, t0 + tw)
                        hn_ps = nps.tile([F, TN], fp, tag="hn")
                        nc.tensor.matmul(out=hn_ps[:], lhsT=w2b_t[l][:],
                                         rhs=aggT[:, sl], start=True, stop=False)
                        nc.tensor.matmul(out=hn_ps[:], lhsT=skw_t[l][:],
                                         rhs=hT[0:F, sl], start=False, stop=True)
                        # x rows 0:64, x^2 rows 64:128
                        xsq = nsb.tile([128, TN], fp, tag="xsq")
                        nc.vector.tensor_scalar(out=xsq[0:F, :], in0=hn_ps[:],
                                                scalar1=skb_t[l][:, 0:1],
                                                scalar2=None, op0=AOT.add)
                        nc.vector.tensor_tensor(out=xsq[F:2 * F, :],
                                                in0=xsq[0:F, :], in1=xsq[0:F, :],
                                                op=AOT.mult)
                        stmu_ps = nps2.tile([1, TN], fp, tag="stmu")
                        stm2_ps = nps2.tile([1, TN], fp, tag="stm2")
                        nc.tensor.matmul(out=stmu_ps[:], lhsT=onesbd_t[:, 0:1],
                                         rhs=xsq[:], start=True, stop=True)
                        nc.tensor.matmul(out=stm2_ps[:], lhsT=onesbd_t[:, 1:2],
                                         rhs=xsq[:], start=True, stop=True)
                        murow = nsb.tile([1, TN], fp, tag="murow")
                        m2row = nsb.tile([1, TN], fp, tag="m2row")
                        srow = nsb.tile([1, TN], fp, tag="srow")
                        trow = nsb.tile([1, TN], fp, tag="trow")
                        nc.vector.tensor_scalar(out=murow[:], in0=stmu_ps[:],
                                                scalar1=1.0 / F, scalar2=None,
                                                op0=AOT.mult)
                        nc.vector.tensor_scalar(out=m2row[:], in0=stm2_ps[:],
                                                scalar1=1.0 / F, scalar2=None,
                                                op0=AOT.mult)
                        # srow = var = (-mu)*mu + m2
                        nc.vector.scalar_tensor_tensor(
                            out=srow[:], in0=murow[:], scalar=-1.0,
                            in1=murow[:], op0=AOT.mult, op1=AOT.mult)
                        nc.vector.tensor_tensor(out=srow[:], in0=srow[:],
                                                in1=m2row[:], op=AOT.add)
                        nc.vector.tensor_scalar(out=srow[:], in0=srow[:],
                                                scalar1=1e-5, scalar2=None,
                                                op0=AOT.add)
                        nc.scalar.activation(out=srow[:], in_=srow[:],
                                             func=ACT.Sqrt)
                        nc.vector.reciprocal(out=srow[:], in_=srow[:])
                        nc.vector.scalar_tensor_tensor(
                            out=trow[:], in0=murow[:], scalar=-1.0,
                            in1=srow[:], op0=AOT.mult, op1=AOT.mult)
                        # broadcast rstd, t to F rows via matmuls
                        sb_ps = nps2.tile([F, TN], fp, tag="sb")
                        tb_ps = nps2.tile([F, TN], fp, tag="tb")
                        nc.tensor.matmul(out=sb_ps[:], lhsT=ones64_t[:],
                                         rhs=srow[:], start=True, stop=True)
                        nc.tensor.matmul(out=tb_ps[:], lhsT=ones64_t[:],
                                         rhs=trow[:], start=True, stop=True)
                        y = nsb.tile([F, TN], fp, tag="y")
                        nc.vector.tensor_tensor(out=y[:], in0=xsq[0:F, :],
                                                in1=sb_ps[:], op=AOT.mult)
                        nc.vector.tensor_tensor(out=y[:], in0=y[:],
                                                in1=tb_ps[:], op=AOT.add)
                        nc.scalar.activation(out=hT[0:F, sl], in_=y[:],
                                             func=ACT.Relu,
                                             bias=lnb_t[l][:, 0:1],
                                             scale=lng_t[l][:, 0:1])

            # ---------------- head ----------------
            nc.sync.dma_start(out=houtT_d[:, :], in_=hT[0:F, :])
            with (
                tc.tile_pool(name="hd_sb", bufs=2) as hsb,
                tc.tile_pool(name="hd_ps", bufs=2, space="PSUM") as hps,
            ):
                for (t0, tw) in ntiles:
                    sl = slice(t0, t0 + tw)
                    z_ps = hps.tile([F, TN], fp, tag="z1")
                    nc.tensor.matmul(out=z_ps[:], lhsT=hw1_t[:],
                                     rhs=hT[0:F, sl], start=True, stop=True)
                    z_sb = hsb.tile([F, TN], fp, tag="z1sb")
                    nc.scalar.activation(out=z_sb[:], in_=z_ps[:], func=ACT.Relu,
                                         bias=hb1_t[:, 0:1])
                    o_ps = hps.tile([POLY, TN], fp, tag="ops")
                    nc.tensor.matmul(out=o_ps[:], lhsT=hw2_t[:],
                                     rhs=z_sb[:], start=True, stop=True)
                    o_sb = hsb.tile([POLY, TN], fp, tag="osb")
                    nc.vector.tensor_scalar(out=o_sb[:], in0=o_ps[:],
                                            scalar1=hb2_t[:, 0:1], scalar2=None,
                                            op0=AOT.add)
                    nc.sync.dma_start(out=outT_d[:, t0:t0 + tw], in_=o_sb[:])

    nc.compile()
    return nc


def _host_arrays(inputs):
    s = np.float32
    enc_w1 = np.asarray(inputs["enc_w1"], s)
    enc_b1 = np.asarray(inputs["enc_b1"], s)
    enc_w2 = np.asarray(inputs["enc_w2"], s)
    enc_b2 = np.asarray(inputs["enc_b2"], s)
    conv_w1 = np.asarray(inputs["conv_w1"], s)  # [L, 130, F]
    conv_b1 = np.asarray(inputs["conv_b1"], s)
    conv_w2 = np.asarray(inputs["conv_w2"], s)
    conv_b2 = np.asarray(inputs["conv_b2"], s)
    skip_w = np.asarray(inputs["skip_w"], s)
    skip_b = np.asarray(inputs["skip_b"], s)
    ln_g = np.asarray(inputs["ln_g"], s)
    ln_b = np.asarray(inputs["ln_b"], s)
    head_w1 = np.asarray(inputs["head_w1"], s)
    head_b1 = np.asarray(inputs["head_b1"], s)
    head_w2 = np.asarray(inputs["head_w2"], s)
    head_b2 = np.asarray(inputs["head_b2"], s)

    import ml_dtypes
    bf = ml_dtypes.bfloat16

    encw1b = np.concatenate([enc_w1, enc_b1[None, :]], axis=0)      # [4, HID]
    wrb1 = np.concatenate([conv_w1[:, 0:F, :], conv_b1[:, None, :]], axis=1)
    wc = np.concatenate([conv_w1[:, F:2 * F, :],
                         np.zeros((L, 1, F), s)], axis=1)
    wrb1wc = np.concatenate([wrb1, wc], axis=2)                     # [L,65,128]
    web = np.zeros((L, 32, F), s)                                   # rows 30:32
    web[:, 30:32, :] = conv_w1[:, 2 * F:2 * F + 2, :]
    web = web.astype(bf)
    w2b = np.concatenate([conv_w2, conv_b2[:, None, :]], axis=1)    # [L,65,F]

    iota = np.tile(np.arange(GW, dtype=s), (128, 1)).astype(bf)
    onesbd = np.zeros((128, 2), s)
    onesbd[0:F, 0] = 1.0
    onesbd[F:2 * F, 1] = 1.0
    ones64 = np.ones((1, F), s)

    return dict(
        iota=iota, onesbd=onesbd, ones64=ones64,
        encw1b=encw1b, encw2=enc_w2, encb2=enc_b2.reshape(F, 1),
        wrb1wc=wrb1wc, web=web, w2b=w2b, skw=skip_w,
        skb=skip_b.reshape(L, F, 1),
        lng=ln_g.reshape(L, F, 1), lnb=ln_b.reshape(L, F, 1),
        hw1=head_w1, hb1=head_b1.reshape(F, 1),
        hw2=head_w2, hb2=head_b2.reshape(POLY, 1),
    )


def _run(inputs, trace=False):
    from concourse import bass_utils

    node_features = np.asarray(inputs["node_features"], np.float32)
    edge_index = np.asarray(inputs["edge_index"])
    edge_features = np.asarray(inputs["edge_features"], np.float32)

    sched, percore = _preprocess(node_features, edge_index, edge_features)
    nc = _build(sched)
    shared = _host_arrays(inputs)

    in_maps = []
    for c in range(NCORES):
        m = dict(shared)
        m["nf1T"] = percore["nf1T"][c]
        m["qidx"] = percore["qidx_w"][c]
        m["rloc"] = percore["rloc"][c]
        m["ohTef"] = percore["ohTef"][c]
        in_maps.append(m)

    res = bass_utils.run_bass_kernel_spmd(
        nc, in_maps, core_ids=list(range(NCORES)), trace=trace)
    outs = res.results
    full = np.concatenate([outs[c]["outT"].T for c in range(NCORES)], axis=0)
    return full[:N], res


def kernel(**inputs) -> np.ndarray:
    out, _ = _run(inputs, trace=False)
    return out
